# revision 35
# baseline (speedup 1.0000x reference)
"""Trainium2 Bass kernel for per-edge dot products (DGL u_dot_v).

score[e] = sum_d h[src[e], d] * h[dst[e], d]   for 640K edges, 10K nodes, D=128.

Strategy (8 NeuronCores, SPMD, one-hot matmul gather — no per-edge DMA):
  - Nodes split into 79 blocks of 128. Edges bucketed by
    (src block, dst block). Src blocks ("rows") are sharded across cores
    (<=10 rows/core); each core runs a STATIC cartesian schedule of
    790 tiles: tile t covers bucket (local row t//79, dst block t%79),
    holding up to 128 edges. ~1% of buckets exceed 128 edges; overflow
    goes to 16 fixed "spill" tiles whose h blocks ship as data.
  - Per tile, the edge membership is encoded host-side as two fp8 one-hot
    matrices [128 nodes x 128 edges]. Per 4-tile group the device runs:
    PE transposed gathers huT/hvT[d,e] (stationary bf16 h block, moving
    fp8 one-hot; the four src gathers of a group batch into ONE matmul
    since they share the row block), ScalarE batch-copy of hvT to SBUF
    bf16 (DVE may read only one PSUM operand), one DVE elementwise mult,
    and a PE ones-matmul reducing over features (emitted LAG groups late
    so PE never stalls; lands at 32-aligned PSUM partitions, 3 groups
    per score tile, drained by ScalarE + DMA).
  - Program structure is input-independent: all data-dependence lives in
    the shipped arrays, so one compile serves any input (SPMD-uniform).
  - Engine instruction streams are kept small (PE ~2.5K instrs) because
    NX IRAM is 128KB/engine; oversized loop bodies thrash instruction
    fetch (measured 5x slowdown at 4x body size).
"""

import sys

import numpy as np

for _p in ("/opt/trn_rl_repo", "/opt/pypackages"):
    if _p not in sys.path:
        sys.path.append(_p)

import ml_dtypes  # noqa: E402

import concourse.mybir as mybir  # noqa: E402
import concourse.tile as tile  # noqa: E402
from concourse import bacc, bass  # noqa: E402
from concourse.bass_utils import run_bass_kernel_spmd  # noqa: E402

N_NODES = 10000
D_FEAT = 128
N_EDGES = 640000
N_CORES = 8
P = 128
NBLK = (N_NODES + P - 1) // P  # 79 node blocks
NR = 10  # src rows per core (7 cores x 10 + 1 core x 9)
TBASE = NR * NBLK  # 790 schedule tiles
SPILL = 18  # spill tiles for buckets > 128 edges
T_TILES = TBASE + SPILL  # 808
CH = 128  # tiles per one-hot DMA chunk (4 MiB merged)
G = 4  # tiles per PSUM group (gather batch / reduce-matmul granularity)
NG = T_TILES // G  # 202 groups
DRAIN = 3  # groups per PSUM score tile (partitions 0/32/64; quadrant 3 is unusable)
ND = (NG + DRAIN - 1) // DRAIN  # drains
LAG = 6  # groups between DVE mult and PE reduce (software pipelining)

_F8 = ml_dtypes.float8_e4m3
_BF16 = ml_dtypes.bfloat16

_BUILT = {}


def build_v4(reps=1, unroll=1, mode="full"):
    """Compile the SPMD per-core program (cached; input-independent).

    v6 pipeline per 4-tile group:
      PE:  huT[d,e] = h_i @ ohs   hvT[d,e] = h_j @ ohd   (transposed gathers,
           stationary = bf16 h block, moving = fp8 one-hot)
      ACT: hvT batch-copy PSUM -> SBUF bf16
      DVE: prodT = huT(PSUM) * hvT(SBUF)  -> SBUF bf16   (one op per group)
      PE:  scores[1, G*128] = ones.T @ prodT  (reduce over features; emitted
           LAG groups late so PE never stalls on DVE; lands at a 32-aligned
           PSUM partition, 4 groups per score tile)
      DVE: drain score tile PSUM -> SBUF;  DMA out per drain.
    """
    key = ("v6", reps, unroll, mode)
    if key in _BUILT:
        return _BUILT[key]
    assert reps == 1 or reps % unroll == 0

    f32 = mybir.dt.float32
    bf16 = mybir.dt.bfloat16
    f8 = mybir.dt.float8e4

    nc = bacc.Bacc("TRN2", target_bir_lowering=False, debug=False)

    ND4 = (ND + 3) // 4
    hfull_d = nc.dram_tensor("hfull", [P, NBLK, D_FEAT], bf16, kind="ExternalInput")
    hsrc_d = nc.dram_tensor("hsrc", [P, NR, D_FEAT], bf16, kind="ExternalInput")
    spb_d = nc.dram_tensor("spb", [P, 2, SPILL, D_FEAT], bf16, kind="ExternalInput")
    # per group g: [:, g, 0, k, :] = src one-hot of tile 4g+k (quad-contiguous
    # for the batched src gather), [:, g, 1, k, :] = dst one-hot.
    ohb_d = nc.dram_tensor("ohb", [P, NG, 2, G, P], f8, kind="ExternalInput")
    out_d = nc.dram_tensor("scores", [ND4, P, 4, G * P], f32, kind="ExternalOutput")

    with tile.TileContext(nc) as tc:
        with (
            tc.tile_pool(name="const", bufs=1) as constp,
            tc.tile_pool(name="oh", bufs=3) as ohp,
            tc.tile_pool(name="work", bufs=LAG + 2) as workp,
            tc.tile_pool(name="gps", bufs=3, space=bass.MemorySpace.PSUM) as gpsp,
            tc.tile_pool(name="rps", bufs=2, space=bass.MemorySpace.PSUM) as rpsp,
            tc.tile_pool(name="outp", bufs=3) as outp,
        ):
            hfull = constp.tile([P, NBLK, D_FEAT], bf16)
            hsrc = constp.tile([P, NR, D_FEAT], bf16)
            spb = constp.tile([P, 2, SPILL, D_FEAT], bf16)
            ones = constp.tile([P, 1], bf16)
            nc.vector.memset(ones[:], 1.0)

            def tile_operands(t):
                if t < TBASE:
                    return hsrc[:, t // NBLK, :], hfull[:, t % NBLK, :]
                return spb[:, 0, t - TBASE, :], spb[:, 1, t - TBASE, :]

            def body():
                nc.sync.dma_start(hfull[:], hfull_d[:])
                nc.sync.dma_start(hsrc[:], hsrc_d[:])
                nc.sync.dma_start(spb[:], spb_d[:])

                state = {"red": None, "scs": None, "pending": []}

                def flush_one():
                    g, prodT = state["pending"].pop(0)
                    q = g % DRAIN
                    if q == 0:
                        red = rpsp.tile([P, G * P], f32, tag="red")
                        state["red"] = red
                    red = state["red"]
                    nc.tensor.matmul(
                        red[32 * q : 32 * q + 1, :],
                        ones[:],
                        prodT[:].rearrange("p g e -> p (g e)"),
                    )
                    if q == DRAIN - 1 or g == NG - 1:
                        d = g // DRAIN
                        if d % 4 == 0:
                            scs = outp.tile([P, 4, G * P], f32, tag="scs")
                            state["scs"] = scs
                        scs = state["scs"]
                        nc.scalar.activation(
                            scs[:, d % 4, :],
                            red[:],
                            mybir.ActivationFunctionType.Copy,
                        )
                        if d % 4 == 3 or g == NG - 1:
                            nc.sync.dma_start(out_d[d // 4], scs[:])

                CHG = CH // G  # groups per chunk
                for ci, c0 in enumerate(range(0, NG, CHG)):
                    n = min(CHG, NG - c0)
                    ohb_sb = ohp.tile([P, CHG, 2, G, P], f8, tag="ohb")
                    nc.sync.dma_start(
                        ohb_sb[:, :n, :, :, :], ohb_d[:, c0 : c0 + n, :, :, :]
                    )
                    for gg in range(n):
                        g = c0 + gg
                        t0 = g * G
                        huT = gpsp.tile([P, G, P], f32, tag="huT")
                        hvT = gpsp.tile([P, G, P], f32, tag="hvT")
                        ops = [tile_operands(t0 + k) for k in range(G)]
                        same_src = t0 + G <= TBASE and (
                            t0 // NBLK == (t0 + G - 1) // NBLK
                        )
                        if same_src:
                            # whole group shares the src row block: one matmul
                            nc.tensor.matmul(
                                huT[:].rearrange("p g e -> p (g e)"),
                                ops[0][0],
                                ohb_sb[:, gg, 0, :, :].rearrange("p g e -> p (g e)"),
                            )
                        else:
                            for k in range(G):
                                nc.tensor.matmul(
                                    huT[:, k, :], ops[k][0], ohb_sb[:, gg, 0, k, :]
                                )
                        for k in range(G):
                            nc.tensor.matmul(
                                hvT[:, k, :], ops[k][1], ohb_sb[:, gg, 1, k, :]
                            )
                        if mode == "gather":
                            continue
                        hvs = workp.tile([P, G, P], bf16, tag="hvs")
                        nc.scalar.activation(
                            hvs[:], hvT[:], mybir.ActivationFunctionType.Copy
                        )
                        prodT = workp.tile([P, G, P], bf16, tag="prodT")
                        nc.vector.tensor_tensor(
                            prodT[:], huT[:], hvs[:], op=mybir.AluOpType.mult
                        )
                        if mode == "nored":
                            continue
                        state["pending"].append((g, prodT))
                        if len(state["pending"]) > LAG:
                            flush_one()
                while state["pending"]:
                    flush_one()

            if reps == 1:
                body()
            else:
                with tc.For_i(0, reps // unroll):
                    for _ in range(unroll):
                        body()

    nc.compile()
    _BUILT[key] = nc
    return nc


def prep_v4(h, src, dst):
    """Host-side packing. Returns (in_maps, core, t, p) or None if the
    spill capacity is exceeded (caller falls back)."""
    E = len(src)
    i = src // P
    j = dst // P
    core = np.minimum(i // NR, N_CORES - 1)
    m = i - core * NR
    t = m * NBLK + j

    key = core * T_TILES + t
    order = np.argsort(key, kind="stable")
    ks = key[order]
    starts = np.r_[0, np.flatnonzero(np.diff(ks)) + 1]
    sizes = np.diff(np.r_[starts, len(ks)])
    rank = np.arange(E) - np.repeat(starts, sizes)
    p = np.empty(E, np.int64)
    p[order] = rank

    # spill: edges ranked >=128 in their bucket get spill tiles, one per
    # (core, bucket, 128-chunk), numbered per core in bucket order.
    ov = p >= P
    if np.any(ov):
        sk = key[ov] * 64 + (p[ov] - P) // P  # unique per spill tile
        uniq, inv = np.unique(sk, return_inverse=True)
        ucore = (uniq // 64) // T_TILES
        # spill index within core
        sidx = np.zeros(len(uniq), np.int64)
        for c in range(N_CORES):
            sel = ucore == c
            if sel.sum() > SPILL:
                return None
            sidx[sel] = np.arange(sel.sum())
        t = t.copy()
        t[ov] = TBASE + sidx[inv]
        p = p.copy()
        p[ov] = (p[ov] - P) % P

    # h layouts
    hpad = np.zeros((NBLK * P, D_FEAT), np.float32)
    hpad[:N_NODES] = h
    h_re = np.ascontiguousarray(
        hpad.reshape(NBLK, P, D_FEAT).transpose(1, 0, 2)
    ).astype(_BF16)  # [128, 79, 128]

    # one-hots: fp8 1.0 == 0x38 bit pattern; group-major layout with the four
    # src one-hots of a group contiguous (for the batched src gather matmul)
    ohb = np.zeros((N_CORES, P, NG, 2, G, P), np.uint8)
    ohb[core, src % P, t // G, 0, t % G, p] = 0x38
    ohb[core, dst % P, t // G, 1, t % G, p] = 0x38

    in_maps = []
    for c in range(N_CORES):
        rows = h_re[:, c * NR : c * NR + NR, :]
        if rows.shape[1] < NR:
            rows = np.concatenate(
                [rows, np.zeros((P, NR - rows.shape[1], D_FEAT), _BF16)], axis=1
            )
        spb = np.zeros((P, 2, SPILL, D_FEAT), _BF16)
        csel = np.flatnonzero((core == c) & (t >= TBASE))
        if len(csel):
            s_ids = t[csel] - TBASE
            spb[:, 0, s_ids, :] = h_re[:, i[csel], :]
            spb[:, 1, s_ids, :] = h_re[:, j[csel], :]
        in_maps.append(
            {
                "hfull": h_re,
                "hsrc": np.ascontiguousarray(rows),
                "spb": spb,
                "ohb": ohb[c].view(_F8),
            }
        )
    return in_maps, core, t, p


def kernel(h, src, dst):
    h = np.asarray(h, dtype=np.float32)
    src = np.asarray(src).astype(np.int64)
    dst = np.asarray(dst).astype(np.int64)

    prep = prep_v4(h, src, dst)
    if prep is None:
        return _kernel_flat(h, src, dst)
    in_maps, core, t, p = prep

    nc = build_v4(reps=1)
    res = run_bass_kernel_spmd(nc, in_maps, list(range(N_CORES)))
    stacked = np.stack([np.asarray(res.results[c]["scores"]) for c in range(N_CORES)])
    g = t // G  # [8, ND4, 128, 4, G*128]
    d = g // DRAIN
    out = stacked[
        core, d // 4, 32 * (g % DRAIN), d % 4, (t % G) * P + p
    ].astype(np.float32)
    return out.reshape(N_EDGES, 1)


# ---------------------------------------------------------------------------
# Fallback: SWDGE dma_gather path (slow but fully general) — used only if
# spill capacity is exceeded (pathological edge distributions).
# ---------------------------------------------------------------------------

TILE_E = 4096
E_PER = N_EDGES // N_CORES


def _edge_tiles(e_per):
    tiles = []
    s = 0
    while s < e_per:
        tl = min(TILE_E, e_per - s)
        tiles.append((s, tl))
        s += tl
    return tiles


def build_flat(e_per, reps=1):
    key = ("flat", e_per, reps)
    if key in _BUILT:
        return _BUILT[key]
    i16 = mybir.dt.int16
    f32 = mybir.dt.float32
    nc = bacc.Bacc("TRN2", target_bir_lowering=False, debug=False, num_swdge_queues=4)
    h_d = nc.dram_tensor("h", [N_NODES, D_FEAT], f32, kind="ExternalInput")
    srcw_d = nc.dram_tensor("srcw", [128, e_per // 16], i16, kind="ExternalInput")
    dstw_d = nc.dram_tensor("dstw", [128, e_per // 16], i16, kind="ExternalInput")
    out_d = nc.dram_tensor("scores", [128, e_per // 128], f32, kind="ExternalOutput")
    with tile.TileContext(nc) as tc:
        with (
            tc.tile_pool(name="const", bufs=1) as constp,
            tc.tile_pool(name="gather", bufs=3) as gpool,
            tc.tile_pool(name="prod", bufs=2) as ppool,
            tc.tile_pool(name="outp", bufs=1) as outp,
        ):
            srcw = constp.tile([128, e_per // 16], i16)
            dstw = constp.tile([128, e_per // 16], i16)
            scores = outp.tile([128, e_per // 128], f32)
            nc.sync.dma_start(srcw[:], srcw_d[:])
            nc.sync.dma_start(dstw[:], dstw_d[:])
            q = 0
            for start, tl in _edge_tiles(e_per) * reps:
                nchunk = tl // 128
                hu = gpool.tile([128, nchunk, D_FEAT], f32, tag="hu")
                hv = gpool.tile([128, nchunk, D_FEAT], f32, tag="hv")
                for dst_t, idx_t in ((hu, srcw), (hv, dstw)):
                    nc.gpsimd.dma_gather(
                        dst_t[:],
                        h_d[:],
                        idx_t[:, start // 16 : (start + tl) // 16],
                        num_idxs=tl,
                        num_idxs_reg=tl,
                        elem_size=D_FEAT,
                        single_packet=False,
                        queue_num=q % 4,
                    )
                    q += 1
                prod = ppool.tile([128, nchunk, D_FEAT], f32)
                nc.vector.tensor_mul(prod[:], hu[:], hv[:])
                nc.vector.tensor_reduce(
                    scores[:, start // 128 : start // 128 + nchunk],
                    prod[:],
                    axis=mybir.AxisListType.X,
                    op=mybir.AluOpType.add,
                )
            nc.sync.dma_start(out_d[:], scores[:])
    nc.compile()
    _BUILT[key] = nc
    return nc


def wrap_idx(ix):
    w = ix.astype(np.int16).reshape(-1, 16).T
    return np.ascontiguousarray(np.tile(w, (8, 1)))


def _kernel_flat(h, src, dst):
    nc = build_flat(E_PER)
    in_maps = []
    for k in range(N_CORES):
        sl = slice(k * E_PER, (k + 1) * E_PER)
        in_maps.append({"h": h, "srcw": wrap_idx(src[sl]), "dstw": wrap_idx(dst[sl])})
    res = run_bass_kernel_spmd(nc, in_maps, list(range(N_CORES)))
    parts = []
    for k in range(N_CORES):
        sc = res.results[k]["scores"]
        parts.append(sc.T.reshape(-1))
    return np.concatenate(parts).astype(np.float32).reshape(N_EDGES, 1)


# revision 36
# speedup vs baseline: 1.0677x; 1.0677x over previous
"""Trainium2 Bass kernel for per-edge dot products (DGL u_dot_v).

score[e] = sum_d h[src[e], d] * h[dst[e], d]   for 640K edges, 10K nodes, D=128.

Strategy (8 NeuronCores, SPMD, one-hot matmul gather — no per-edge DMA):
  - Nodes split into 79 blocks of 128. Edges bucketed by
    (src block, dst block). Src blocks ("rows") are sharded across cores
    (<=10 rows/core); each core runs a STATIC cartesian schedule of
    790 tiles: tile t covers bucket (local row t//79, dst block t%79),
    holding up to 128 edges. ~1% of buckets exceed 128 edges; overflow
    goes to 16 fixed "spill" tiles whose h blocks ship as data.
  - Per tile, the edge membership is encoded host-side as two fp8 one-hot
    matrices [128 nodes x 128 edges]. Per 4-tile group the device runs:
    PE transposed gathers huT/hvT[d,e] (stationary bf16 h block, moving
    fp8 one-hot; the four src gathers of a group batch into ONE matmul
    since they share the row block), ScalarE batch-copy of hvT to SBUF
    bf16 (DVE may read only one PSUM operand), one DVE elementwise mult,
    and a PE ones-matmul reducing over features (emitted LAG groups late
    so PE never stalls; lands at 32-aligned PSUM partitions, 3 groups
    per score tile, drained by ScalarE + DMA).
  - Program structure is input-independent: all data-dependence lives in
    the shipped arrays, so one compile serves any input (SPMD-uniform).
  - Engine instruction streams are kept small (PE ~2.5K instrs) because
    NX IRAM is 128KB/engine; oversized loop bodies thrash instruction
    fetch (measured 5x slowdown at 4x body size).
"""

import sys

import numpy as np

for _p in ("/opt/trn_rl_repo", "/opt/pypackages"):
    if _p not in sys.path:
        sys.path.append(_p)

import ml_dtypes  # noqa: E402

import concourse.mybir as mybir  # noqa: E402
import concourse.tile as tile  # noqa: E402
from concourse import bacc, bass  # noqa: E402
from concourse.bass_utils import run_bass_kernel_spmd  # noqa: E402

N_NODES = 10000
D_FEAT = 128
N_EDGES = 640000
N_CORES = 8
P = 128
NBLK = (N_NODES + P - 1) // P  # 79 node blocks
NR = 10  # src rows per core (7 cores x 10 + 1 core x 9)
TBASE = NR * NBLK  # 790 schedule tiles
SPILL = 18  # spill tiles for buckets > 128 edges
T_TILES = TBASE + SPILL  # 808
CH = 128  # tiles per one-hot DMA chunk (4 MiB merged)
G = 4  # tiles per PSUM group (gather batch / reduce-matmul granularity)
NG = T_TILES // G  # 202 groups
DRAIN = 3  # groups per PSUM score tile (partitions 0/32/64; quadrant 3 is unusable)
ND = (NG + DRAIN - 1) // DRAIN  # drains
LAG = 6  # groups between DVE mult and PE reduce (software pipelining)
DRAIN_SPLIT = False  # alternate score drains between ScalarE and DVE

_F8 = ml_dtypes.float8_e4m3
_BF16 = ml_dtypes.bfloat16

_BUILT = {}


def build_v4(reps=1, unroll=1, mode="full"):
    """Compile the SPMD per-core program (cached; input-independent).

    v6 pipeline per 4-tile group:
      PE:  huT[d,e] = h_i @ ohs   hvT[d,e] = h_j @ ohd   (transposed gathers,
           stationary = bf16 h block, moving = fp8 one-hot)
      ACT: hvT batch-copy PSUM -> SBUF bf16
      DVE: prodT = huT(PSUM) * hvT(SBUF)  -> SBUF bf16   (one op per group)
      PE:  scores[1, G*128] = ones.T @ prodT  (reduce over features; emitted
           LAG groups late so PE never stalls on DVE; lands at a 32-aligned
           PSUM partition, 4 groups per score tile)
      DVE: drain score tile PSUM -> SBUF;  DMA out per drain.
    """
    key = ("v6", reps, unroll, mode)
    if key in _BUILT:
        return _BUILT[key]
    assert reps == 1 or reps % unroll == 0

    f32 = mybir.dt.float32
    bf16 = mybir.dt.bfloat16
    f8 = mybir.dt.float8e4

    nc = bacc.Bacc("TRN2", target_bir_lowering=False, debug=False)

    ND4 = (ND + 3) // 4
    hfull_d = nc.dram_tensor("hfull", [P, NBLK, D_FEAT], bf16, kind="ExternalInput")
    hsrc_d = nc.dram_tensor("hsrc", [P, NR, D_FEAT], bf16, kind="ExternalInput")
    spb_d = nc.dram_tensor("spb", [P, 2, SPILL, D_FEAT], bf16, kind="ExternalInput")
    # per group g: [:, g, 0, k, :] = src one-hot of tile 4g+k (quad-contiguous
    # for the batched src gather), [:, g, 1, k, :] = dst one-hot.
    ohb_d = nc.dram_tensor("ohb", [P, NG, 2, G, P], f8, kind="ExternalInput")
    out_d = nc.dram_tensor("scores", [ND4, P, 4, G * P], f32, kind="ExternalOutput")

    with tile.TileContext(nc) as tc:
        with (
            tc.tile_pool(name="const", bufs=1) as constp,
            tc.tile_pool(name="oh", bufs=3) as ohp,
            tc.tile_pool(name="work", bufs=LAG + 2) as workp,
            tc.tile_pool(name="gps", bufs=3, space=bass.MemorySpace.PSUM) as gpsp,
            tc.tile_pool(name="rps", bufs=2, space=bass.MemorySpace.PSUM) as rpsp,
            tc.tile_pool(name="outp", bufs=3) as outp,
        ):
            hfull = constp.tile([P, NBLK, D_FEAT], bf16)
            hsrc = constp.tile([P, NR, D_FEAT], bf16)
            spb = constp.tile([P, 2, SPILL, D_FEAT], bf16)
            ones = constp.tile([P, 1], bf16)
            nc.vector.memset(ones[:], 1.0)

            def tile_operands(t):
                if t < TBASE:
                    return hsrc[:, t // NBLK, :], hfull[:, t % NBLK, :]
                return spb[:, 0, t - TBASE, :], spb[:, 1, t - TBASE, :]

            def body():
                nc.sync.dma_start(hfull[:], hfull_d[:])
                nc.sync.dma_start(hsrc[:], hsrc_d[:])
                nc.sync.dma_start(spb[:], spb_d[:])

                state = {"red": None, "scs": None, "pending": []}

                def flush_one():
                    g, prodT = state["pending"].pop(0)
                    q = g % DRAIN
                    if q == 0:
                        red = rpsp.tile([P, G * P], f32, tag="red")
                        state["red"] = red
                    red = state["red"]
                    nc.tensor.matmul(
                        red[32 * q : 32 * q + 1, :],
                        ones[:],
                        prodT[:].rearrange("p g e -> p (g e)"),
                    )
                    if q == DRAIN - 1 or g == NG - 1:
                        d = g // DRAIN
                        if d % 4 == 0:
                            scs = outp.tile([P, 4, G * P], f32, tag="scs")
                            state["scs"] = scs
                        scs = state["scs"]
                        if DRAIN_SPLIT and d % 2 == 1:
                            nc.vector.tensor_copy(scs[:, d % 4, :], red[:])
                        else:
                            nc.scalar.activation(
                                scs[:, d % 4, :],
                                red[:],
                                mybir.ActivationFunctionType.Copy,
                            )
                        if d % 4 == 3 or g == NG - 1:
                            nc.sync.dma_start(out_d[d // 4], scs[:])

                CHG = CH // G  # groups per chunk
                for ci, c0 in enumerate(range(0, NG, CHG)):
                    n = min(CHG, NG - c0)
                    ohb_sb = ohp.tile([P, CHG, 2, G, P], f8, tag="ohb")
                    nc.sync.dma_start(
                        ohb_sb[:, :n, :, :, :], ohb_d[:, c0 : c0 + n, :, :, :]
                    )
                    for gg in range(n):
                        g = c0 + gg
                        t0 = g * G
                        huT = gpsp.tile([P, G, P], f32, tag="huT")
                        hvT = gpsp.tile([P, G, P], f32, tag="hvT")
                        ops = [tile_operands(t0 + k) for k in range(G)]
                        same_src = t0 + G <= TBASE and (
                            t0 // NBLK == (t0 + G - 1) // NBLK
                        )
                        if same_src:
                            # whole group shares the src row block: one matmul
                            nc.tensor.matmul(
                                huT[:].rearrange("p g e -> p (g e)"),
                                ops[0][0],
                                ohb_sb[:, gg, 0, :, :].rearrange("p g e -> p (g e)"),
                            )
                        else:
                            for k in range(G):
                                nc.tensor.matmul(
                                    huT[:, k, :], ops[k][0], ohb_sb[:, gg, 0, k, :]
                                )
                        for k in range(G):
                            nc.tensor.matmul(
                                hvT[:, k, :], ops[k][1], ohb_sb[:, gg, 1, k, :]
                            )
                        if mode == "gather":
                            continue
                        hvs = workp.tile([P, G, P], bf16, tag="hvs")
                        nc.scalar.activation(
                            hvs[:], hvT[:], mybir.ActivationFunctionType.Copy
                        )
                        prodT = workp.tile([P, G, P], bf16, tag="prodT")
                        nc.vector.tensor_tensor(
                            prodT[:], huT[:], hvs[:], op=mybir.AluOpType.mult
                        )
                        if mode == "nored":
                            continue
                        state["pending"].append((g, prodT))
                        if len(state["pending"]) > LAG:
                            flush_one()
                while state["pending"]:
                    flush_one()

            if reps == 1:
                body()
            else:
                with tc.For_i(0, reps // unroll):
                    for _ in range(unroll):
                        body()

    nc.compile()
    _BUILT[key] = nc
    return nc


def prep_v4(h, src, dst):
    """Host-side packing. Returns (in_maps, core, t, p) or None if the
    spill capacity is exceeded (caller falls back)."""
    E = len(src)
    i = src // P
    j = dst // P
    core = np.minimum(i // NR, N_CORES - 1)
    m = i - core * NR
    t = m * NBLK + j

    key = core * T_TILES + t
    order = np.argsort(key, kind="stable")
    ks = key[order]
    starts = np.r_[0, np.flatnonzero(np.diff(ks)) + 1]
    sizes = np.diff(np.r_[starts, len(ks)])
    rank = np.arange(E) - np.repeat(starts, sizes)
    p = np.empty(E, np.int64)
    p[order] = rank

    # spill: edges ranked >=128 in their bucket get spill tiles, one per
    # (core, bucket, 128-chunk), numbered per core in bucket order.
    ov = p >= P
    if np.any(ov):
        sk = key[ov] * 64 + (p[ov] - P) // P  # unique per spill tile
        uniq, inv = np.unique(sk, return_inverse=True)
        ucore = (uniq // 64) // T_TILES
        # spill index within core
        sidx = np.zeros(len(uniq), np.int64)
        for c in range(N_CORES):
            sel = ucore == c
            if sel.sum() > SPILL:
                return None
            sidx[sel] = np.arange(sel.sum())
        t = t.copy()
        t[ov] = TBASE + sidx[inv]
        p = p.copy()
        p[ov] = (p[ov] - P) % P

    # h layouts
    hpad = np.zeros((NBLK * P, D_FEAT), np.float32)
    hpad[:N_NODES] = h
    h_re = np.ascontiguousarray(
        hpad.reshape(NBLK, P, D_FEAT).transpose(1, 0, 2)
    ).astype(_BF16)  # [128, 79, 128]

    # one-hots: fp8 1.0 == 0x38 bit pattern; group-major layout with the four
    # src one-hots of a group contiguous (for the batched src gather matmul)
    ohb = np.zeros((N_CORES, P, NG, 2, G, P), np.uint8)
    ohb[core, src % P, t // G, 0, t % G, p] = 0x38
    ohb[core, dst % P, t // G, 1, t % G, p] = 0x38

    in_maps = []
    for c in range(N_CORES):
        rows = h_re[:, c * NR : c * NR + NR, :]
        if rows.shape[1] < NR:
            rows = np.concatenate(
                [rows, np.zeros((P, NR - rows.shape[1], D_FEAT), _BF16)], axis=1
            )
        spb = np.zeros((P, 2, SPILL, D_FEAT), _BF16)
        csel = np.flatnonzero((core == c) & (t >= TBASE))
        if len(csel):
            s_ids = t[csel] - TBASE
            spb[:, 0, s_ids, :] = h_re[:, i[csel], :]
            spb[:, 1, s_ids, :] = h_re[:, j[csel], :]
        in_maps.append(
            {
                "hfull": h_re,
                "hsrc": np.ascontiguousarray(rows),
                "spb": spb,
                "ohb": ohb[c].view(_F8),
            }
        )
    return in_maps, core, t, p


def kernel(h, src, dst):
    h = np.asarray(h, dtype=np.float32)
    src = np.asarray(src).astype(np.int64)
    dst = np.asarray(dst).astype(np.int64)

    prep = prep_v4(h, src, dst)
    if prep is None:
        return _kernel_flat(h, src, dst)
    in_maps, core, t, p = prep

    nc = build_v4(reps=1)
    res = run_bass_kernel_spmd(nc, in_maps, list(range(N_CORES)))
    stacked = np.stack([np.asarray(res.results[c]["scores"]) for c in range(N_CORES)])
    g = t // G  # [8, ND4, 128, 4, G*128]
    d = g // DRAIN
    out = stacked[
        core, d // 4, 32 * (g % DRAIN), d % 4, (t % G) * P + p
    ].astype(np.float32)
    return out.reshape(N_EDGES, 1)


# ---------------------------------------------------------------------------
# Fallback: SWDGE dma_gather path (slow but fully general) — used only if
# spill capacity is exceeded (pathological edge distributions).
# ---------------------------------------------------------------------------

TILE_E = 4096
E_PER = N_EDGES // N_CORES


def _edge_tiles(e_per):
    tiles = []
    s = 0
    while s < e_per:
        tl = min(TILE_E, e_per - s)
        tiles.append((s, tl))
        s += tl
    return tiles


def build_flat(e_per, reps=1):
    key = ("flat", e_per, reps)
    if key in _BUILT:
        return _BUILT[key]
    i16 = mybir.dt.int16
    f32 = mybir.dt.float32
    nc = bacc.Bacc("TRN2", target_bir_lowering=False, debug=False, num_swdge_queues=4)
    h_d = nc.dram_tensor("h", [N_NODES, D_FEAT], f32, kind="ExternalInput")
    srcw_d = nc.dram_tensor("srcw", [128, e_per // 16], i16, kind="ExternalInput")
    dstw_d = nc.dram_tensor("dstw", [128, e_per // 16], i16, kind="ExternalInput")
    out_d = nc.dram_tensor("scores", [128, e_per // 128], f32, kind="ExternalOutput")
    with tile.TileContext(nc) as tc:
        with (
            tc.tile_pool(name="const", bufs=1) as constp,
            tc.tile_pool(name="gather", bufs=3) as gpool,
            tc.tile_pool(name="prod", bufs=2) as ppool,
            tc.tile_pool(name="outp", bufs=1) as outp,
        ):
            srcw = constp.tile([128, e_per // 16], i16)
            dstw = constp.tile([128, e_per // 16], i16)
            scores = outp.tile([128, e_per // 128], f32)
            nc.sync.dma_start(srcw[:], srcw_d[:])
            nc.sync.dma_start(dstw[:], dstw_d[:])
            q = 0
            for start, tl in _edge_tiles(e_per) * reps:
                nchunk = tl // 128
                hu = gpool.tile([128, nchunk, D_FEAT], f32, tag="hu")
                hv = gpool.tile([128, nchunk, D_FEAT], f32, tag="hv")
                for dst_t, idx_t in ((hu, srcw), (hv, dstw)):
                    nc.gpsimd.dma_gather(
                        dst_t[:],
                        h_d[:],
                        idx_t[:, start // 16 : (start + tl) // 16],
                        num_idxs=tl,
                        num_idxs_reg=tl,
                        elem_size=D_FEAT,
                        single_packet=False,
                        queue_num=q % 4,
                    )
                    q += 1
                prod = ppool.tile([128, nchunk, D_FEAT], f32)
                nc.vector.tensor_mul(prod[:], hu[:], hv[:])
                nc.vector.tensor_reduce(
                    scores[:, start // 128 : start // 128 + nchunk],
                    prod[:],
                    axis=mybir.AxisListType.X,
                    op=mybir.AluOpType.add,
                )
            nc.sync.dma_start(out_d[:], scores[:])
    nc.compile()
    _BUILT[key] = nc
    return nc


def wrap_idx(ix):
    w = ix.astype(np.int16).reshape(-1, 16).T
    return np.ascontiguousarray(np.tile(w, (8, 1)))


def _kernel_flat(h, src, dst):
    nc = build_flat(E_PER)
    in_maps = []
    for k in range(N_CORES):
        sl = slice(k * E_PER, (k + 1) * E_PER)
        in_maps.append({"h": h, "srcw": wrap_idx(src[sl]), "dstw": wrap_idx(dst[sl])})
    res = run_bass_kernel_spmd(nc, in_maps, list(range(N_CORES)))
    parts = []
    for k in range(N_CORES):
        sc = res.results[k]["scores"]
        parts.append(sc.T.reshape(-1))
    return np.concatenate(parts).astype(np.float32).reshape(N_EDGES, 1)
